# revision 7
# baseline (speedup 1.0000x reference)
"""Trainium2 Bass kernel for 2-layer BiLSTM + classifier (nn_BiLSTM_45234595561814).

Strategy (8 NeuronCores, single SPMD launch, no collectives):
  - Each core q owns a 64-token window W_q = [64q, 64q+64) of T=512, FULL batch
    (B=64), and runs BOTH directions as 2 independent interleaved chains
    (hides the ~2us per-step cross-engine dependency chain).
  - Sequence parallelism via truncated warmup: LSTM state decays ~0.5/step, so
    a chain zero-initialized WARM steps before its window converges to the
    exact state (err ~ WARM * 2^-WARM; ~1e-12 at WARM=48).  Layer-0 chains
    span [window-W, window+64+W) so layer-1 warmups are fed locally -> the
    (L0,L1) cascade self-warms; no cross-core exchange anywhere.
  - One-tanh trick: i,f,o weight rows pre-scaled by 0.5 so sigmoid(z) =
    0.5*(1+tanh(z/2)) needs only tanh -> ONE ACT op for all 4 gates.
    State kept doubled (C=2c, hh=2h); cell update is 3 scalar_tensor_tensor
    DVE ops + 1 for hh.  Whh pre-scaled by extra 0.5 to absorb hh=2h.
  - L0 input projection fused into the per-step PSUM accumulation (K=65 with
    a ones row carrying the bias).  L1 projection precomputed into DRAM
    (bf16) and streamed back; accumulated into gate PSUM via identity-matmul.
  - Pad tokens (outside [0,512)) handled exactly: x/ones rows zero keep state
    at 0 through leading pads; an L1 control row drives the i-gate preact to
    -30000 on pad tokens so pad xg1 cannot perturb state.
  - Classifier is fully local; final GEMM emitted transposed (tokens on
    partitions) so the output DMA is contiguous.

kernel(**inputs) takes the FULL inputs and returns the FULL [64,512,64] f32
output.  Self-contained: hardcodes all shapes; no sibling imports.
"""

import os

import numpy as np
import ml_dtypes

import concourse.bass as bass
import concourse.mybir as mybir
import concourse.tile as tile
from concourse import bacc
from concourse.bass_utils import run_bass_kernel_spmd

bf16 = ml_dtypes.bfloat16
F32, BF16 = mybir.dt.float32, mybir.dt.bfloat16
AluOp = mybir.AluOpType
ACT_TANH = mybir.ActivationFunctionType.Tanh
ACT_RELU = mybir.ActivationFunctionType.Relu

H = 128          # rnn size
B = 64           # batch
T = 512          # seq len
D = 64           # input size
NC = 8           # cores
WIN = T // NC    # tokens per core window = 64
WARM = int(os.environ.get("BILSTM_WARM", "48"))
SPAN0 = WIN + 2 * WARM   # L0 chain steps (slots)
SPAN1 = WIN + WARM       # L1 chain steps
PADKILL = -30000.0

_CACHE = {}


def _build_program():
    nc = bacc.Bacc(None, target_bir_lowering=False)

    # ---------------- I/O declarations ----------------
    ei = lambda name, shape, dt=BF16: nc.dram_tensor(name, shape, dt, kind="ExternalInput")
    xaug = ei("xaug", [D + 1, SPAN0 * B])          # rows 0..63 x.T, row 64 valid-ones
    ctl1 = ei("ctl1", [2, SPAN0 * B])              # row0 valid, row1 padkill indicator
    wihT0 = {d: ei(f"wihT0{d}", [D + 1, 4 * H]) for d in "fb"}
    whhT0 = {d: ei(f"whhT0{d}", [H, 4 * H]) for d in "fb"}
    whhT1 = {d: ei(f"whhT1{d}", [H, 4 * H]) for d in "fb"}
    wih1Ta = {d: ei(f"wih1Ta{d}", [H, 4 * H]) for d in "fb"}   # y0f K-tile
    wih1Tb = {d: ei(f"wih1Tb{d}", [H, 4 * H]) for d in "fb"}   # y0b K-tile
    ctlT1 = {d: ei(f"ctlT1{d}", [2, 4 * H]) for d in "fb"}     # bias row + padkill row
    idn = ei("idn", [H, H])
    w1Ta = ei("w1Ta", [H, 2 * H])   # (0.5*W1).T rows 0:128  -> [128, 256]
    w1Tb = ei("w1Tb", [H, 2 * H])   # rows 128:256
    b1row = ei("b1row", [1, 2 * H])
    w2Ta = ei("w2Ta", [H, D])       # W2.T rows 0:128 -> [128, 64]
    w2Tb = ei("w2Tb", [H, D])
    b2row = ei("b2row", [1, D])
    out = nc.dram_tensor("out", [WIN * B, D], F32, kind="ExternalOutput")

    with tile.TileContext(nc) as tc:
        with tc.tile_pool(name="singles", bufs=1) as singles, \
             tc.tile_pool(name="state", bufs=1) as state, \
             tc.tile_pool(name="tpool", bufs=3) as tpool, \
             tc.tile_pool(name="vpool", bufs=3) as vpool, \
             tc.tile_pool(name="xg1fetch", bufs=3) as xg1fetch, \
             tc.tile_pool(name="stage", bufs=3) as stage_pool, \
             tc.tile_pool(name="clssb", bufs=3) as clssb, \
             tc.tile_pool(name="psA", bufs=2, space="PSUM") as psA, \
             tc.tile_pool(name="psB", bufs=2, space="PSUM") as psB, \
             tc.tile_pool(name="psP", bufs=2, space="PSUM") as psP, \
             tc.tile_pool(name="psC", bufs=2, space="PSUM") as psC, \
             tc.tile_pool(name="dram", bufs=1, space="DRAM") as dram:

            # ---------------- load constants ----------------
            def load(src, shape, dt=BF16):
                t = singles.tile(shape, dt, name=src.name, tag=src.name)
                nc.sync.dma_start(out=t[:], in_=src[:])
                return t

            xaug_t = load(xaug, [D + 1, SPAN0 * B])
            ctl1_t = load(ctl1, [2, SPAN0 * B])
            wihT0_t = {d: load(wihT0[d], [D + 1, 4 * H]) for d in "fb"}
            whhT0_t = {d: load(whhT0[d], [H, 4 * H]) for d in "fb"}
            whhT1_t = {d: load(whhT1[d], [H, 4 * H]) for d in "fb"}
            wih1Ta_t = {d: load(wih1Ta[d], [H, 4 * H]) for d in "fb"}
            wih1Tb_t = {d: load(wih1Tb[d], [H, 4 * H]) for d in "fb"}
            ctlT1_t = {d: load(ctlT1[d], [2, 4 * H]) for d in "fb"}
            idn_t = load(idn, [H, H])
            w1Ta_t = load(w1Ta, [H, 2 * H])
            w1Tb_t = load(w1Tb, [H, 2 * H])
            b1row_t = load(b1row, [1, 2 * H])
            w2Ta_t = load(w2Ta, [H, D])
            w2Tb_t = load(w2Tb, [H, D])
            b2row_t = load(b2row, [1, D])

            # ---------------- persistent state ----------------
            y0 = {d: state.tile([H, SPAN0 * B], BF16, name=f"y0{d}", tag=f"y0{d}") for d in "fb"}
            y1 = {d: state.tile([H, SPAN1 * B], BF16, name=f"y1{d}", tag=f"y1{d}") for d in "fb"}
            C0 = {d: state.tile([H, B], F32, name=f"C0{d}", tag=f"C0{d}") for d in "fb"}
            C1 = {d: state.tile([H, B], F32, name=f"C1{d}", tag=f"C1{d}") for d in "fb"}
            h00 = state.tile([H, B], BF16, name="h00", tag="h00")
            nc.vector.memset(h00[:], 0.0)
            for d in "fb":
                nc.vector.memset(C0[d][:], 0.0)
                nc.vector.memset(C1[d][:], 0.0)

            xg1_dram = {d: dram.tile([H, SPAN1 * 4 * B], BF16, name=f"xg1d{d}", tag=f"xg1d{d}") for d in "fb"}

            # ---------------- generic LSTM step ----------------
            def lstm_step(tag, whh_t, Ct, hprev, yout_slice, gate_accum):
                """One cell step.  gate_accum(g_ps) adds the input projection
                into the gate psum tile after the 4 Whh matmuls (start) wrote it.
                Gate cols: [i | f | g | o] * B."""
                ps_pool = psA if tag.endswith("f") else psB
                g_ps = ps_pool.tile([H, 4 * B], F32, name="g" + tag, tag="g" + tag[-1])
                gate_accum(g_ps, whh_t, hprev)
                t_t = tpool.tile([H, 4 * B], F32, name="t" + tag, tag="t" + tag)
                nc.scalar.activation(t_t[:], g_ps[:], ACT_TANH)
                ti, tf = t_t[:, 0:B], t_t[:, B:2 * B]
                tg, to = t_t[:, 2 * B:3 * B], t_t[:, 3 * B:4 * B]
                A_t = vpool.tile([H, B], F32, name="A" + tag, tag="A" + tag)
                Bv_t = vpool.tile([H, B], F32, name="B" + tag, tag="B" + tag)
                nc.vector.scalar_tensor_tensor(A_t[:], tf, 1.0, Ct[:], AluOp.add, AluOp.mult)
                nc.vector.scalar_tensor_tensor(Bv_t[:], ti, 1.0, tg, AluOp.add, AluOp.mult)
                nc.vector.scalar_tensor_tensor(Ct[:], A_t[:], 0.5, Bv_t[:], AluOp.mult, AluOp.add)
                tc_t = vpool.tile([H, B], F32, name="c" + tag, tag="c" + tag)
                nc.scalar.activation(tc_t[:], Ct[:], ACT_TANH, scale=0.5)
                nc.vector.scalar_tensor_tensor(yout_slice, to, 1.0, tc_t, AluOp.add, AluOp.mult)

            # ---------------- layer 0 (fused input projection) ----------------
            def l0_accum(dirn, slot):
                def fn(g_ps, whh_t, hprev):
                    for g in range(4):
                        nc.tensor.matmul(g_ps[:, g * B:(g + 1) * B],
                                         whh_t[:, g * H:(g + 1) * H], hprev,
                                         start=True, stop=False)
                        nc.tensor.matmul(g_ps[:, g * B:(g + 1) * B],
                                         wihT0_t[dirn][:, g * H:(g + 1) * H],
                                         xaug_t[:, slot * B:(slot + 1) * B],
                                         start=False, stop=True)
                return fn

            for step in range(SPAN0):
                sf = step                 # fwd slot, ascending
                sb = SPAN0 - 1 - step     # bwd slot, descending
                hp_f = h00[:] if step == 0 else y0["f"][:, (sf - 1) * B:sf * B]
                hp_b = h00[:] if step == 0 else y0["b"][:, (sb + 1) * B:(sb + 2) * B]
                lstm_step("0f", whhT0_t["f"], C0["f"], hp_f,
                          y0["f"][:, sf * B:(sf + 1) * B], l0_accum("f", sf))
                lstm_step("0b", whhT0_t["b"], C0["b"], hp_b,
                          y0["b"][:, sb * B:(sb + 1) * B], l0_accum("b", sb))

            # ---------------- layer-1 projection -> DRAM ----------------
            # xg1_d covers slots [lo, lo+SPAN1); col layout per slot: [i|f|g|o]*B
            CH = 512                      # psum cols per chunk = 8 slots
            SLOTS_PER_CH = CH // B
            NCH = SPAN1 * B // CH         # 14 chunks
            # chain f consumes slots [0, SPAN1); chain b consumes [WARM, SPAN0)
            proj_lo = {"f": 0, "b": WARM}

            def l1_proj_chunk(dirn, j):
                lo = proj_lo[dirn]
                col0 = (lo + j * SLOTS_PER_CH) * B          # into y0/ctl tiles
                st = stage_pool.tile([H, SLOTS_PER_CH * 4 * B], BF16, name="st", tag="st")
                for g in range(4):
                    p = psP.tile([H, CH], F32, name="pp", tag="pp")
                    nc.tensor.matmul(p[:], wih1Ta_t[dirn][:, g * H:(g + 1) * H],
                                     y0["f"][:, col0:col0 + CH], start=True, stop=False)
                    nc.tensor.matmul(p[:], wih1Tb_t[dirn][:, g * H:(g + 1) * H],
                                     y0["b"][:, col0:col0 + CH], start=False, stop=False)
                    nc.tensor.matmul(p[:], ctlT1_t[dirn][:, g * H:(g + 1) * H],
                                     ctl1_t[:, col0:col0 + CH], start=False, stop=True)
                    # scatter gate g into per-slot interleaved layout
                    st3 = st[:].rearrange("h (s c) -> h s c", s=SLOTS_PER_CH)
                    nc.any.tensor_copy(st3[:, :, g * B:(g + 1) * B],
                                       p[:].rearrange("h (s b) -> h s b", b=B))
                nc.sync.dma_start(
                    out=xg1_dram[dirn][:, j * SLOTS_PER_CH * 4 * B:(j + 1) * SLOTS_PER_CH * 4 * B],
                    in_=st[:])

            # emit chunks roughly in readiness order (f's late chunks finish
            # first because chain b writes its low slots last, and vice versa)
            for j in range(NCH - 1, -1, -1):
                l1_proj_chunk("f", j)
            for j in range(NCH):
                l1_proj_chunk("b", j)

            # ---------------- layer 1 recurrence ----------------
            def l1_accum(dirn, xg_slice):
                def fn(g_ps, whh_t, hprev):
                    nc.tensor.matmul(g_ps[:], idn_t[:], xg_slice, start=True, stop=False)
                    for g in range(4):
                        nc.tensor.matmul(g_ps[:, g * B:(g + 1) * B],
                                         whh_t[:, g * H:(g + 1) * H], hprev,
                                         start=False, stop=(g == 3),
                                         skip_group_check=True)
                return fn

            # xg1 fetch tiles: 8 slots per fetch, chain f ascending, chain b descending
            NFETCH = SPAN1 // SLOTS_PER_CH
            fet = {"f": [None] * NFETCH, "b": [None] * NFETCH}

            def get_fetch(dirn, k):
                if fet[dirn][k] is None:
                    ft = xg1fetch.tile([H, SLOTS_PER_CH * 4 * B], BF16, name="x" + dirn, tag="x" + dirn)
                    if dirn == "f":     # fetch k covers local idx [8k, 8k+8)
                        c0 = k * SLOTS_PER_CH * 4 * B
                    else:               # fetch k covers local idx [SPAN1-8(k+1), SPAN1-8k)
                        c0 = (SPAN1 - (k + 1) * SLOTS_PER_CH) * 4 * B
                    nc.sync.dma_start(out=ft[:], in_=xg1_dram[dirn][:, c0:c0 + SLOTS_PER_CH * 4 * B])
                    fet[dirn][k] = ft
                return fet[dirn][k]

            for step in range(SPAN1):
                # fwd: slot = step, local idx within xg1f = step
                kf = step // SLOTS_PER_CH
                ff = get_fetch("f", kf)
                xs_f = ff[:, (step % SLOTS_PER_CH) * 4 * B:((step % SLOTS_PER_CH) + 1) * 4 * B]
                # bwd: slot = SPAN0-1-step, local idx within xg1b = SPAN1-1-step
                kb = step // SLOTS_PER_CH
                fb_ = get_fetch("b", kb)
                within_b = (SLOTS_PER_CH - 1) - (step % SLOTS_PER_CH)
                xs_b = fb_[:, within_b * 4 * B:(within_b + 1) * 4 * B]

                hp_f = h00[:] if step == 0 else y1["f"][:, (step - 1) * B:step * B]
                hp_b = h00[:] if step == 0 else y1["b"][:, (SPAN1 - step) * B:(SPAN1 - step + 1) * B]
                lstm_step("1f", whhT1_t["f"], C1["f"], hp_f,
                          y1["f"][:, step * B:(step + 1) * B], l1_accum("f", xs_f))
                lstm_step("1b", whhT1_t["b"], C1["b"], hp_b,
                          y1["b"][:, (SPAN1 - 1 - step) * B:(SPAN1 - step) * B],
                          l1_accum("b", xs_b))

            # ---------------- classifier (window slots only) ----------------
            # window tokens: slot s in [WARM, WARM+WIN)
            #   y1f idx = s        -> cols [WARM*B, (WARM+WIN)*B)
            #   y1b idx = s - WARM -> cols [0, WIN*B)
            # ones: ctl1 row0 cols [WARM*B ...)
            NTOK = WIN * B                      # 4096 columns
            h1 = [clssb.tile([H, NTOK], BF16, name="h1a", tag="h1a", bufs=1), clssb.tile([H, NTOK], BF16, name="h1b", tag="h1b", bufs=1)]
            for c0 in range(0, NTOK, CH):
                for m in range(2):
                    p = psC.tile([H, CH], F32, name="pc", tag="pc")
                    nc.tensor.matmul(p[:], w1Ta_t[:, m * H:(m + 1) * H],
                                     y1["f"][:, WARM * B + c0:WARM * B + c0 + CH],
                                     start=True, stop=False)
                    nc.tensor.matmul(p[:], w1Tb_t[:, m * H:(m + 1) * H],
                                     y1["b"][:, c0:c0 + CH], start=False, stop=False)
                    nc.tensor.matmul(p[:], b1row_t[:, m * H:(m + 1) * H],
                                     ctl1_t[0:1, WARM * B + c0:WARM * B + c0 + CH],
                                     start=False, stop=True)
                    nc.scalar.activation(h1[m][:, c0:c0 + CH], p[:], ACT_RELU)

            # final GEMM transposed: out[tok, d] (tokens on partitions)
            for c0 in range(0, NTOK, H):
                p = psC.tile([H, D], F32, name="po", tag="pc")
                nc.tensor.matmul(p[:], h1[0][:, c0:c0 + H], w2Ta_t[:], start=True, stop=False)
                nc.tensor.matmul(p[:], h1[1][:, c0:c0 + H], w2Tb_t[:], start=False, stop=False)
                nc.tensor.matmul(p[:], ctl1_t[0:1, WARM * B + c0:WARM * B + c0 + H],
                                 b2row_t[:], start=False, stop=True)
                o_t = clssb.tile([H, D], F32, name="ot", tag="ot")
                nc.scalar.activation(o_t[:], p[:], ACT_TANH)
                nc.sync.dma_start(out=out[c0:c0 + H, :], in_=o_t[:])

    nc.compile()
    return nc


# ======================= host side =======================

def _prep_weights(inp):
    """Returns dict of np arrays shared by all cores (bf16)."""
    sr = np.full((4 * H, 1), 0.5, np.float32)
    sr[2 * H:3 * H] = 1.0
    w = {}
    for d, tag in (("f", "0"), ("b", "1")):
        Wih, Whh = inp[f"Wih0{tag}"], inp[f"Whh0{tag}"]
        bias = inp[f"bih0{tag}"] + inp[f"bhh0{tag}"]
        w[f"wihT0{d}"] = np.concatenate([Wih * sr, (bias[:, None] * sr)], 1).T.astype(bf16)
        w[f"whhT0{d}"] = (Whh * sr * 0.5).T.astype(bf16)
        Wih1, Whh1 = inp[f"Wih1{tag}"], inp[f"Whh1{tag}"]
        bias1 = (inp[f"bih1{tag}"] + inp[f"bhh1{tag}"])[None, :] * sr.T
        w[f"whhT1{d}"] = (Whh1 * sr * 0.5).T.astype(bf16)
        w[f"wih1Ta{d}"] = (Wih1[:, :H] * sr * 0.5).T.astype(bf16)
        w[f"wih1Tb{d}"] = (Wih1[:, H:] * sr * 0.5).T.astype(bf16)
        padkill = np.zeros((1, 4 * H), np.float32)
        padkill[0, :H] = PADKILL          # kill i-gate preact on pad tokens
        w[f"ctlT1{d}"] = np.concatenate([bias1, padkill], 0).astype(bf16)
    w["idn"] = np.eye(H, dtype=np.float32).astype(bf16)
    w["w1Ta"] = (0.5 * inp["W1"][:, :H]).T.astype(bf16)
    w["w1Tb"] = (0.5 * inp["W1"][:, H:]).T.astype(bf16)
    w["b1row"] = inp["b1"][None, :].astype(bf16)
    w["w2Ta"] = inp["W2"][:, :H].T.astype(bf16)
    w["w2Tb"] = inp["W2"][:, H:].T.astype(bf16)
    w["b2row"] = inp["b2"][None, :].astype(bf16)
    return w


def _per_core_inputs(x, q):
    """x: [B, T, D] f32.  Builds xaug [65, SPAN0*B] and ctl1 [2, SPAN0*B]."""
    t0 = WIN * q - WARM
    xaug = np.zeros((D + 1, SPAN0 * B), np.float32)
    ctl = np.zeros((2, SPAN0 * B), np.float32)
    for s in range(SPAN0):
        t = t0 + s
        sl = slice(s * B, (s + 1) * B)
        if 0 <= t < T:
            xaug[:D, sl] = x[:, t, :].T
            xaug[D, sl] = 1.0
            ctl[0, sl] = 1.0
        else:
            ctl[1, sl] = 1.0
    return xaug.astype(bf16), ctl.astype(bf16)


def _get_program():
    if "nc" not in _CACHE:
        _CACHE["nc"] = _build_program()
    return _CACHE["nc"]


def _run(inputs, trace=False):
    inp = {k: np.asarray(v) for k, v in inputs.items()}
    nc = _get_program()
    w = _prep_weights(inp)
    x = inp["x"].astype(np.float32)
    in_maps = []
    for q in range(NC):
        xaug, ctl = _per_core_inputs(x, q)
        m = dict(w)
        m["xaug"] = xaug
        m["ctl1"] = ctl
        in_maps.append(m)
    res = run_bass_kernel_spmd(nc, in_maps, list(range(NC)), trace=trace)
    outp = np.zeros((B, T, D), np.float32)
    for q in range(NC):
        o = res.results[q]["out"].reshape(WIN, B, D)        # [tok, b, d]
        outp[:, WIN * q:WIN * (q + 1), :] = o.transpose(1, 0, 2)
    return outp, res


def kernel(**inputs):
    out, _ = _run(inputs, trace=False)
    return out


# revision 11
# speedup vs baseline: 1.4736x; 1.4736x over previous
"""Trainium2 Bass kernel for 2-layer BiLSTM + classifier (nn_BiLSTM_45234595561814).

Strategy (8 NeuronCores, single SPMD launch, no collectives):
  - Each core q owns a 64-token window W_q = [64q, 64q+64) of T=512, FULL batch
    (B=64), and runs BOTH directions as 2 independent interleaved chains
    (hides the ~2us per-step cross-engine dependency chain).
  - Sequence parallelism via truncated warmup: LSTM state decays ~0.5/step, so
    a chain zero-initialized WARM steps before its window converges to the
    exact state (err ~ WARM * 2^-WARM; ~1e-12 at WARM=48).  Layer-0 chains
    span [window-W, window+64+W) so layer-1 warmups are fed locally -> the
    (L0,L1) cascade self-warms; no cross-core exchange anywhere.
  - One-tanh trick: i,f,o weight rows pre-scaled by 0.5 so sigmoid(z) =
    0.5*(1+tanh(z/2)) needs only tanh -> ONE ACT op for all 4 gates.
    State kept doubled (C=2c, hh=2h); cell update is 3 scalar_tensor_tensor
    DVE ops + 1 for hh.  Whh pre-scaled by extra 0.5 to absorb hh=2h.
  - L0 input projection fused into the per-step PSUM accumulation (K=65 with
    a ones row carrying the bias).  L1 projection precomputed into DRAM
    (bf16) and streamed back; accumulated into gate PSUM via identity-matmul.
  - Pad tokens (outside [0,512)) handled exactly: x/ones rows zero keep state
    at 0 through leading pads; an L1 control row drives the i-gate preact to
    -30000 on pad tokens so pad xg1 cannot perturb state.
  - Classifier is fully local; final GEMM emitted transposed (tokens on
    partitions) so the output DMA is contiguous.

kernel(**inputs) takes the FULL inputs and returns the FULL [64,512,64] f32
output.  Self-contained: hardcodes all shapes; no sibling imports.
"""

import os

import numpy as np
import ml_dtypes

import concourse.bass as bass
import concourse.mybir as mybir
import concourse.tile as tile
from concourse import bacc
from concourse.bass_utils import run_bass_kernel_spmd

bf16 = ml_dtypes.bfloat16
F32, BF16 = mybir.dt.float32, mybir.dt.bfloat16
AluOp = mybir.AluOpType
ACT_TANH = mybir.ActivationFunctionType.Tanh
ACT_RELU = mybir.ActivationFunctionType.Relu

H = 128          # rnn size
B = 64           # batch
T = 512          # seq len
D = 64           # input size
NC = 8           # cores
WIN = T // NC    # tokens per core window = 64
WARM = int(os.environ.get("BILSTM_WARM", "32"))
SPAN0 = WIN + 2 * WARM   # L0 chain steps (slots)
SPAN1 = WIN + WARM       # L1 chain steps
PADKILL = -30000.0

_CACHE = {}


def _build_program():
    nc = bacc.Bacc(None, target_bir_lowering=False)

    # ---------------- I/O declarations ----------------
    ei = lambda name, shape, dt=BF16: nc.dram_tensor(name, shape, dt, kind="ExternalInput")
    xaug = ei("xaug", [D + 1, SPAN0 * B])          # rows 0..63 x.T, row 64 valid-ones
    ctl1 = ei("ctl1", [2, SPAN0 * B])              # row0 valid, row1 padkill indicator
    wihT0 = {d: ei(f"wihT0{d}", [D + 1, 4 * H]) for d in "fb"}
    whhT0 = {d: ei(f"whhT0{d}", [H, 4 * H]) for d in "fb"}
    whhT1 = {d: ei(f"whhT1{d}", [H, 4 * H]) for d in "fb"}
    wih1Ta = {d: ei(f"wih1Ta{d}", [H, 4 * H]) for d in "fb"}   # y0f K-tile
    wih1Tb = {d: ei(f"wih1Tb{d}", [H, 4 * H]) for d in "fb"}   # y0b K-tile
    ctlT1 = {d: ei(f"ctlT1{d}", [2, 4 * H]) for d in "fb"}     # bias row + padkill row
    idn = ei("idn", [H, H])
    w1Ta = ei("w1Ta", [H, 2 * H])   # (0.5*W1).T rows 0:128  -> [128, 256]
    w1Tb = ei("w1Tb", [H, 2 * H])   # rows 128:256
    b1row = ei("b1row", [1, 2 * H])
    w2Ta = ei("w2Ta", [H, D])       # W2.T rows 0:128 -> [128, 64]
    w2Tb = ei("w2Tb", [H, D])
    b2row = ei("b2row", [1, D])
    out = nc.dram_tensor("out", [WIN * B, D], F32, kind="ExternalOutput")

    with tile.TileContext(nc) as tc:
        with tc.tile_pool(name="singles", bufs=1) as singles, \
             tc.tile_pool(name="state", bufs=1) as state, \
             tc.tile_pool(name="tpool", bufs=3) as tpool, \
             tc.tile_pool(name="vpool", bufs=3) as vpool, \
             tc.tile_pool(name="xg1fetch", bufs=3) as xg1fetch, \
             tc.tile_pool(name="stage", bufs=3) as stage_pool, \
             tc.tile_pool(name="clssb", bufs=3) as clssb, \
             tc.tile_pool(name="psA", bufs=2, space="PSUM") as psA, \
             tc.tile_pool(name="psB", bufs=2, space="PSUM") as psB, \
             tc.tile_pool(name="psP", bufs=2, space="PSUM") as psP, \
             tc.tile_pool(name="dram", bufs=1, space="DRAM") as dram:

            # ---------------- load constants ----------------
            def load(src, shape, dt=BF16):
                t = singles.tile(shape, dt, name=src.name, tag=src.name)
                nc.sync.dma_start(out=t[:], in_=src[:])
                return t

            xaug_t = load(xaug, [D + 1, SPAN0 * B])
            ctl1_t = load(ctl1, [2, SPAN0 * B])
            wihT0_t = {d: load(wihT0[d], [D + 1, 4 * H]) for d in "fb"}
            whhT0_t = {d: load(whhT0[d], [H, 4 * H]) for d in "fb"}
            whhT1_t = {d: load(whhT1[d], [H, 4 * H]) for d in "fb"}
            wih1Ta_t = {d: load(wih1Ta[d], [H, 4 * H]) for d in "fb"}
            wih1Tb_t = {d: load(wih1Tb[d], [H, 4 * H]) for d in "fb"}
            ctlT1_t = {d: load(ctlT1[d], [2, 4 * H]) for d in "fb"}
            idn_t = load(idn, [H, H])
            w1Ta_t = load(w1Ta, [H, 2 * H])
            w1Tb_t = load(w1Tb, [H, 2 * H])
            b1row_t = load(b1row, [1, 2 * H])
            w2Ta_t = load(w2Ta, [H, D])
            w2Tb_t = load(w2Tb, [H, D])
            b2row_t = load(b2row, [1, D])

            # ---------------- persistent state ----------------
            y0 = {d: state.tile([H, SPAN0 * B], BF16, name=f"y0{d}", tag=f"y0{d}") for d in "fb"}
            y1 = {d: state.tile([H, SPAN1 * B], BF16, name=f"y1{d}", tag=f"y1{d}") for d in "fb"}
            C0 = {d: state.tile([H, B], F32, name=f"C0{d}", tag=f"C0{d}") for d in "fb"}
            C1 = {d: state.tile([H, B], F32, name=f"C1{d}", tag=f"C1{d}") for d in "fb"}
            h00 = state.tile([H, B], BF16, name="h00", tag="h00")
            nc.vector.memset(h00[:], 0.0)
            for d in "fb":
                nc.vector.memset(C0[d][:], 0.0)
                nc.vector.memset(C1[d][:], 0.0)

            xg1_dram = {d: dram.tile([H, SPAN1 * 4 * B], BF16, name=f"xg1d{d}", tag=f"xg1d{d}") for d in "fb"}

            # ---------------- generic LSTM step ----------------
            # Gate col order in psum/t-tile: [o | i | f | g]*B; cell state
            # C=2c lives in t-tile cols 4B:5B (written by the PREVIOUS step's
            # c-update into THIS step's tile, so (1+ti)*tg and (1+tf)*C fuse
            # into one scalar_tensor_tensor over [i|f] x [g|C]).
            def lstm_prefetch(tag, inproj, first=False):
                ps_pool = psA if tag.endswith("f") else psB
                g_ps = ps_pool.tile([H, 4 * B], F32, name="g" + tag, tag="g" + tag[-1],
                                    bufs=3)
                inproj(g_ps)
                t_t = tpool.tile([H, 5 * B], F32, name="t" + tag, tag="t" + tag, bufs=4)
                if first:
                    nc.vector.memset(t_t[:, 4 * B:5 * B], 0.0)
                return g_ps, t_t

            def lstm_step(tag, whh_t, hprev, yout_slice, cur, nxt):
                g_ps, T = cur
                Tn = nxt[1]
                for g in range(4):
                    nc.tensor.matmul(g_ps[:, g * B:(g + 1) * B],
                                     whh_t[:, g * H:(g + 1) * H], hprev,
                                     start=False, stop=(g == 3),
                                     skip_group_check=True)
                nc.scalar.activation(T[:, B:4 * B], g_ps[:, B:4 * B], ACT_TANH)
                nc.scalar.activation(T[:, 0:B], g_ps[:, 0:B], ACT_TANH)
                scr = vpool.tile([H, 2 * B], F32, name="s" + tag, tag="s" + tag)
                # scr = [(1+ti)*tg | (1+tf)*C] = [Bv | A]
                nc.vector.scalar_tensor_tensor(scr[:], T[:, B:3 * B], 1.0,
                                               T[:, 3 * B:5 * B], AluOp.add, AluOp.mult)
                nc.vector.scalar_tensor_tensor(Tn[:, 4 * B:5 * B], scr[:, B:2 * B], 0.5,
                                               scr[:, 0:B], AluOp.mult, AluOp.add)
                tc_t = vpool.tile([H, B], F32, name="c" + tag, tag="c" + tag)
                nc.scalar.activation(tc_t[:], Tn[:, 4 * B:5 * B], ACT_TANH, scale=0.5)
                nc.vector.scalar_tensor_tensor(yout_slice, T[:, 0:B], 1.0, tc_t[:],
                                               AluOp.add, AluOp.mult)

            # ---------------- layer 0 (fused input projection) ----------------
            def l0_inproj(dirn, slot):
                def fn(g_ps):
                    for g in range(4):
                        nc.tensor.matmul(g_ps[:, g * B:(g + 1) * B],
                                         wihT0_t[dirn][:, g * H:(g + 1) * H],
                                         xaug_t[:, slot * B:(slot + 1) * B],
                                         start=True, stop=False,
                                         skip_group_check=True)
                return fn

            pend0 = {}
            for step in range(SPAN0 + 1):
                sf = step                 # fwd slot, ascending
                sb = SPAN0 - 1 - step     # bwd slot, descending
                if step < SPAN0:
                    pend0[("f", step)] = lstm_prefetch("0f", l0_inproj("f", sf), first=(step == 0))
                    pend0[("b", step)] = lstm_prefetch("0b", l0_inproj("b", sb), first=(step == 0))
                else:
                    pend0[("f", step)] = lstm_prefetch("0f", lambda ps: None)
                    pend0[("b", step)] = lstm_prefetch("0b", lambda ps: None)
                if step >= 1:
                    pf, pb = step - 1, SPAN0 - step
                    hp_f = h00[:] if pf == 0 else y0["f"][:, (pf - 1) * B:pf * B]
                    hp_b = h00[:] if pf == 0 else y0["b"][:, (pb + 1) * B:(pb + 2) * B]
                    lstm_step("0f", whhT0_t["f"], hp_f,
                              y0["f"][:, pf * B:(pf + 1) * B],
                              pend0.pop(("f", pf)), pend0[("f", step)])
                    lstm_step("0b", whhT0_t["b"], hp_b,
                              y0["b"][:, pb * B:(pb + 1) * B],
                              pend0.pop(("b", step - 1)), pend0[("b", step)])

            # ---------------- layer-1 projection -> DRAM ----------------
            # xg1_d covers slots [lo, lo+SPAN1); col layout per slot: [i|f|g|o]*B
            CH = 512                      # psum cols per chunk = 8 slots
            SLOTS_PER_CH = CH // B
            NCH = SPAN1 * B // CH         # 14 chunks
            # chain f consumes slots [0, SPAN1); chain b consumes [WARM, SPAN0)
            proj_lo = {"f": 0, "b": WARM}

            def l1_proj_chunk(dirn, j):
                lo = proj_lo[dirn]
                col0 = (lo + j * SLOTS_PER_CH) * B          # into y0/ctl tiles
                st = stage_pool.tile([H, SLOTS_PER_CH * 4 * B], BF16, name="st", tag="st")
                for g in range(4):
                    p = psP.tile([H, CH], F32, name="pp", tag="pp")
                    nc.tensor.matmul(p[:], wih1Ta_t[dirn][:, g * H:(g + 1) * H],
                                     y0["f"][:, col0:col0 + CH], start=True, stop=False)
                    nc.tensor.matmul(p[:], wih1Tb_t[dirn][:, g * H:(g + 1) * H],
                                     y0["b"][:, col0:col0 + CH], start=False, stop=False)
                    nc.tensor.matmul(p[:], ctlT1_t[dirn][:, g * H:(g + 1) * H],
                                     ctl1_t[:, col0:col0 + CH], start=False, stop=True)
                    # scatter gate g into per-slot interleaved layout
                    st3 = st[:].rearrange("h (s c) -> h s c", s=SLOTS_PER_CH)
                    nc.any.tensor_copy(st3[:, :, g * B:(g + 1) * B],
                                       p[:].rearrange("h (s b) -> h s b", b=B))
                nc.sync.dma_start(
                    out=xg1_dram[dirn][:, j * SLOTS_PER_CH * 4 * B:(j + 1) * SLOTS_PER_CH * 4 * B],
                    in_=st[:])

            # ---------------- layer 1 recurrence (proj interleaved) ----------------
            # xg1 fetch tiles: 8 slots per fetch, chain f ascending, chain b descending
            NFETCH = SPAN1 // SLOTS_PER_CH
            fet = {"f": [None] * NFETCH, "b": [None] * NFETCH}

            def get_fetch(dirn, k):
                if fet[dirn][k] is None:
                    ft = xg1fetch.tile([H, SLOTS_PER_CH * 4 * B], BF16, name="x" + dirn, tag="x" + dirn)
                    if dirn == "f":     # fetch k covers local idx [8k, 8k+8)
                        c0 = k * SLOTS_PER_CH * 4 * B
                    else:               # fetch k covers local idx [SPAN1-8(k+1), SPAN1-8k)
                        c0 = (SPAN1 - (k + 1) * SLOTS_PER_CH) * 4 * B
                    nc.sync.dma_start(out=ft[:], in_=xg1_dram[dirn][:, c0:c0 + SLOTS_PER_CH * 4 * B])
                    fet[dirn][k] = ft
                return fet[dirn][k]

            def l1_inproj(xs):
                def fn(g_ps):
                    nc.tensor.matmul(g_ps[:], idn_t[:], xs, start=True, stop=False,
                                     skip_group_check=True)
                return fn

            # chunks needed first: f ascending from 0, b descending from NCH-1
            l1_proj_chunk("f", 0)
            l1_proj_chunk("b", NCH - 1)

            pend1 = {}
            for step in range(SPAN1 + 1):
                if step < SPAN1:
                    if step % SLOTS_PER_CH == 0:
                        k = step // SLOTS_PER_CH
                        if k + 1 < NCH:
                            l1_proj_chunk("f", k + 1)
                        if NCH - 2 - k >= 0:
                            l1_proj_chunk("b", NCH - 2 - k)
                    kf = step // SLOTS_PER_CH
                    ff = get_fetch("f", kf)
                    xs_f = ff[:, (step % SLOTS_PER_CH) * 4 * B:((step % SLOTS_PER_CH) + 1) * 4 * B]
                    fb_ = get_fetch("b", kf)
                    within_b = (SLOTS_PER_CH - 1) - (step % SLOTS_PER_CH)
                    xs_b = fb_[:, within_b * 4 * B:(within_b + 1) * 4 * B]
                    pend1[("f", step)] = lstm_prefetch("1f", l1_inproj(xs_f), first=(step == 0))
                    pend1[("b", step)] = lstm_prefetch("1b", l1_inproj(xs_b), first=(step == 0))
                else:
                    pend1[("f", step)] = lstm_prefetch("1f", lambda ps: None)
                    pend1[("b", step)] = lstm_prefetch("1b", lambda ps: None)
                if step >= 1:
                    pf = step - 1
                    pb = SPAN1 - 1 - pf
                    hp_f = h00[:] if pf == 0 else y1["f"][:, (pf - 1) * B:pf * B]
                    hp_b = h00[:] if pf == 0 else y1["b"][:, (pb + 1) * B:(pb + 2) * B]
                    lstm_step("1f", whhT1_t["f"], hp_f,
                              y1["f"][:, pf * B:(pf + 1) * B],
                              pend1.pop(("f", pf)), pend1[("f", step)])
                    lstm_step("1b", whhT1_t["b"], hp_b,
                              y1["b"][:, pb * B:(pb + 1) * B],
                              pend1.pop(("b", pf)), pend1[("b", step)])

            # ---------------- classifier (window slots only) ----------------
            # window tokens: slot s in [WARM, WARM+WIN)
            #   y1f idx = s        -> cols [WARM*B, (WARM+WIN)*B)
            #   y1b idx = s - WARM -> cols [0, WIN*B)
            # ones: ctl1 row0 cols [WARM*B ...)
            NTOK = WIN * B                      # 4096 columns
            h1 = [clssb.tile([H, NTOK], BF16, name="h1a", tag="h1a", bufs=1), clssb.tile([H, NTOK], BF16, name="h1b", tag="h1b", bufs=1)]
            for c0 in range(0, NTOK, CH):
                for m in range(2):
                    p = psP.tile([H, CH], F32, name="pc", tag="pp")
                    nc.tensor.matmul(p[:], w1Ta_t[:, m * H:(m + 1) * H],
                                     y1["f"][:, WARM * B + c0:WARM * B + c0 + CH],
                                     start=True, stop=False)
                    nc.tensor.matmul(p[:], w1Tb_t[:, m * H:(m + 1) * H],
                                     y1["b"][:, c0:c0 + CH], start=False, stop=False)
                    nc.tensor.matmul(p[:], b1row_t[:, m * H:(m + 1) * H],
                                     ctl1_t[0:1, WARM * B + c0:WARM * B + c0 + CH],
                                     start=False, stop=True)
                    nc.scalar.activation(h1[m][:, c0:c0 + CH], p[:], ACT_RELU)

            # final GEMM transposed: out[tok, d] (tokens on partitions)
            for c0 in range(0, NTOK, H):
                p = psP.tile([H, D], F32, name="po", tag="pp")
                nc.tensor.matmul(p[:], h1[0][:, c0:c0 + H], w2Ta_t[:], start=True, stop=False)
                nc.tensor.matmul(p[:], h1[1][:, c0:c0 + H], w2Tb_t[:], start=False, stop=False)
                nc.tensor.matmul(p[:], ctl1_t[0:1, WARM * B + c0:WARM * B + c0 + H],
                                 b2row_t[:], start=False, stop=True)
                o_t = clssb.tile([H, D], F32, name="ot", tag="ot")
                nc.scalar.activation(o_t[:], p[:], ACT_TANH)
                nc.sync.dma_start(out=out[c0:c0 + H, :], in_=o_t[:])

    nc.compile()
    return nc


# ======================= host side =======================

def _prep_weights(inp):
    """Returns dict of np arrays shared by all cores (bf16).

    Gate row-blocks reordered from reference [i,f,g,o] to device [o,i,f,g];
    i,f,o rows scaled 0.5 (one-tanh trick)."""
    H_ = H
    sr = np.full((4 * H_, 1), 0.5, np.float32)
    sr[2 * H_:3 * H_] = 1.0

    def reorder(a):           # rows [i,f,g,o] -> [o,i,f,g]
        return np.concatenate([a[3 * H_:], a[:H_], a[H_:2 * H_], a[2 * H_:3 * H_]], 0)

    w = {}
    for d, tag in (("f", "0"), ("b", "1")):
        Wih, Whh = inp[f"Wih0{tag}"], inp[f"Whh0{tag}"]
        bias = inp[f"bih0{tag}"] + inp[f"bhh0{tag}"]
        w[f"wihT0{d}"] = reorder(np.concatenate([Wih * sr, (bias[:, None] * sr)], 1)).T.astype(bf16)
        w[f"whhT0{d}"] = reorder(Whh * sr * 0.5).T.astype(bf16)
        Wih1, Whh1 = inp[f"Wih1{tag}"], inp[f"Whh1{tag}"]
        bias1 = reorder((inp[f"bih1{tag}"] + inp[f"bhh1{tag}"])[:, None] * sr).T
        w[f"whhT1{d}"] = reorder(Whh1 * sr * 0.5).T.astype(bf16)
        w[f"wih1Ta{d}"] = reorder(Wih1[:, :H] * sr * 0.5).T.astype(bf16)
        w[f"wih1Tb{d}"] = reorder(Wih1[:, H:] * sr * 0.5).T.astype(bf16)
        padkill = np.zeros((1, 4 * H), np.float32)
        padkill[0, H:2 * H] = PADKILL      # i-gate block (device order [o,i,f,g])
        w[f"ctlT1{d}"] = np.concatenate([bias1, padkill], 0).astype(bf16)
    w["idn"] = np.eye(H, dtype=np.float32).astype(bf16)
    w["w1Ta"] = (0.5 * inp["W1"][:, :H]).T.astype(bf16)
    w["w1Tb"] = (0.5 * inp["W1"][:, H:]).T.astype(bf16)
    w["b1row"] = inp["b1"][None, :].astype(bf16)
    w["w2Ta"] = inp["W2"][:, :H].T.astype(bf16)
    w["w2Tb"] = inp["W2"][:, H:].T.astype(bf16)
    w["b2row"] = inp["b2"][None, :].astype(bf16)
    return w


def _per_core_inputs(x, q):
    """x: [B, T, D] f32.  Builds xaug [65, SPAN0*B] and ctl1 [2, SPAN0*B]."""
    t0 = WIN * q - WARM
    xaug = np.zeros((D + 1, SPAN0 * B), np.float32)
    ctl = np.zeros((2, SPAN0 * B), np.float32)
    for s in range(SPAN0):
        t = t0 + s
        sl = slice(s * B, (s + 1) * B)
        if 0 <= t < T:
            xaug[:D, sl] = x[:, t, :].T
            xaug[D, sl] = 1.0
            ctl[0, sl] = 1.0
        else:
            ctl[1, sl] = 1.0
    return xaug.astype(bf16), ctl.astype(bf16)


def _get_program():
    if "nc" not in _CACHE:
        _CACHE["nc"] = _build_program()
    return _CACHE["nc"]


def _run(inputs, trace=False):
    inp = {k: np.asarray(v) for k, v in inputs.items()}
    nc = _get_program()
    w = _prep_weights(inp)
    x = inp["x"].astype(np.float32)
    in_maps = []
    for q in range(NC):
        xaug, ctl = _per_core_inputs(x, q)
        m = dict(w)
        m["xaug"] = xaug
        m["ctl1"] = ctl
        in_maps.append(m)
    res = run_bass_kernel_spmd(nc, in_maps, list(range(NC)), trace=trace)
    outp = np.zeros((B, T, D), np.float32)
    for q in range(NC):
        o = res.results[q]["out"].reshape(WIN, B, D)        # [tok, b, d]
        outp[:, WIN * q:WIN * (q + 1), :] = o.transpose(1, 0, 2)
    return outp, res


def kernel(**inputs):
    out, _ = _run(inputs, trace=False)
    return out


# revision 12
# speedup vs baseline: 1.4745x; 1.0006x over previous
"""Trainium2 Bass kernel for 2-layer BiLSTM + classifier (nn_BiLSTM_45234595561814).

Strategy (8 NeuronCores, single SPMD launch, no collectives):
  - Each core q owns a 64-token window W_q = [64q, 64q+64) of T=512, FULL batch
    (B=64), and runs BOTH directions as 2 independent interleaved chains
    (hides the ~2us per-step cross-engine dependency chain).
  - Sequence parallelism via truncated warmup: LSTM state decays ~0.5/step, so
    a chain zero-initialized WARM steps before its window converges to the
    exact state (err ~ WARM * 2^-WARM; ~1e-12 at WARM=48).  Layer-0 chains
    span [window-W, window+64+W) so layer-1 warmups are fed locally -> the
    (L0,L1) cascade self-warms; no cross-core exchange anywhere.
  - One-tanh trick: i,f,o weight rows pre-scaled by 0.5 so sigmoid(z) =
    0.5*(1+tanh(z/2)) needs only tanh -> ONE ACT op for all 4 gates.
    State kept doubled (C=2c, hh=2h); cell update is 3 scalar_tensor_tensor
    DVE ops + 1 for hh.  Whh pre-scaled by extra 0.5 to absorb hh=2h.
  - L0 input projection fused into the per-step PSUM accumulation (K=65 with
    a ones row carrying the bias).  L1 projection precomputed into DRAM
    (bf16) and streamed back; accumulated into gate PSUM via identity-matmul.
  - Pad tokens (outside [0,512)) handled exactly: x/ones rows zero keep state
    at 0 through leading pads; an L1 control row drives the i-gate preact to
    -30000 on pad tokens so pad xg1 cannot perturb state.
  - Classifier is fully local; final GEMM emitted transposed (tokens on
    partitions) so the output DMA is contiguous.

kernel(**inputs) takes the FULL inputs and returns the FULL [64,512,64] f32
output.  Self-contained: hardcodes all shapes; no sibling imports.
"""

import os

import numpy as np
import ml_dtypes

import concourse.bass as bass
import concourse.mybir as mybir
import concourse.tile as tile
from concourse import bacc
from concourse.bass_utils import run_bass_kernel_spmd

bf16 = ml_dtypes.bfloat16
F32, BF16 = mybir.dt.float32, mybir.dt.bfloat16
AluOp = mybir.AluOpType
ACT_TANH = mybir.ActivationFunctionType.Tanh
ACT_RELU = mybir.ActivationFunctionType.Relu

H = 128          # rnn size
B = 64           # batch
T = 512          # seq len
D = 64           # input size
NC = 8           # cores
WIN = T // NC    # tokens per core window = 64
WARM = int(os.environ.get("BILSTM_WARM", "32"))
SPAN0 = WIN + 2 * WARM   # L0 chain steps (slots)
SPAN1 = WIN + WARM       # L1 chain steps
PADKILL = -30000.0

_CACHE = {}


def _build_program():
    nc = bacc.Bacc(None, target_bir_lowering=False)

    # ---------------- I/O declarations ----------------
    ei = lambda name, shape, dt=BF16: nc.dram_tensor(name, shape, dt, kind="ExternalInput")
    xaug = ei("xaug", [D + 1, SPAN0 * B])          # rows 0..63 x.T, row 64 valid-ones
    ctl1 = ei("ctl1", [2, SPAN0 * B])              # row0 valid, row1 padkill indicator
    wihT0 = {d: ei(f"wihT0{d}", [D + 1, 4 * H]) for d in "fb"}
    whhT0 = {d: ei(f"whhT0{d}", [H, 4 * H]) for d in "fb"}
    whhT1 = {d: ei(f"whhT1{d}", [H, 4 * H]) for d in "fb"}
    wih1Ta = {d: ei(f"wih1Ta{d}", [H, 4 * H]) for d in "fb"}   # y0f K-tile
    wih1Tb = {d: ei(f"wih1Tb{d}", [H, 4 * H]) for d in "fb"}   # y0b K-tile
    ctlT1 = {d: ei(f"ctlT1{d}", [2, 4 * H]) for d in "fb"}     # bias row + padkill row
    idn = ei("idn", [H, H])
    w1Ta = ei("w1Ta", [H, 2 * H])   # (0.5*W1).T rows 0:128  -> [128, 256]
    w1Tb = ei("w1Tb", [H, 2 * H])   # rows 128:256
    b1row = ei("b1row", [1, 2 * H])
    w2Ta = ei("w2Ta", [H, D])       # W2.T rows 0:128 -> [128, 64]
    w2Tb = ei("w2Tb", [H, D])
    b2row = ei("b2row", [1, D])
    out = nc.dram_tensor("out", [WIN * B, D], F32, kind="ExternalOutput")

    with tile.TileContext(nc) as tc:
        with tc.tile_pool(name="singles", bufs=1) as singles, \
             tc.tile_pool(name="state", bufs=1) as state, \
             tc.tile_pool(name="tpool", bufs=3) as tpool, \
             tc.tile_pool(name="vpool", bufs=3) as vpool, \
             tc.tile_pool(name="xg1fetch", bufs=3) as xg1fetch, \
             tc.tile_pool(name="stage", bufs=3) as stage_pool, \
             tc.tile_pool(name="clssb", bufs=3) as clssb, \
             tc.tile_pool(name="psA", bufs=2, space="PSUM") as psA, \
             tc.tile_pool(name="psB", bufs=2, space="PSUM") as psB, \
             tc.tile_pool(name="psP", bufs=2, space="PSUM") as psP, \
             tc.tile_pool(name="dram", bufs=1, space="DRAM") as dram:

            # ---------------- load constants ----------------
            def load(src, shape, dt=BF16):
                t = singles.tile(shape, dt, name=src.name, tag=src.name)
                nc.sync.dma_start(out=t[:], in_=src[:])
                return t

            xaug_t = load(xaug, [D + 1, SPAN0 * B])
            ctl1_t = load(ctl1, [2, SPAN0 * B])
            wihT0_t = {d: load(wihT0[d], [D + 1, 4 * H]) for d in "fb"}
            whhT0_t = {d: load(whhT0[d], [H, 4 * H]) for d in "fb"}
            whhT1_t = {d: load(whhT1[d], [H, 4 * H]) for d in "fb"}
            wih1Ta_t = {d: load(wih1Ta[d], [H, 4 * H]) for d in "fb"}
            wih1Tb_t = {d: load(wih1Tb[d], [H, 4 * H]) for d in "fb"}
            ctlT1_t = {d: load(ctlT1[d], [2, 4 * H]) for d in "fb"}
            idn_t = load(idn, [H, H])
            w1Ta_t = load(w1Ta, [H, 2 * H])
            w1Tb_t = load(w1Tb, [H, 2 * H])
            b1row_t = load(b1row, [1, 2 * H])
            w2Ta_t = load(w2Ta, [H, D])
            w2Tb_t = load(w2Tb, [H, D])
            b2row_t = load(b2row, [1, D])

            # ---------------- persistent state ----------------
            y0 = {d: state.tile([H, SPAN0 * B], BF16, name=f"y0{d}", tag=f"y0{d}") for d in "fb"}
            y1 = {d: state.tile([H, SPAN1 * B], BF16, name=f"y1{d}", tag=f"y1{d}") for d in "fb"}
            C0 = {d: state.tile([H, B], F32, name=f"C0{d}", tag=f"C0{d}") for d in "fb"}
            C1 = {d: state.tile([H, B], F32, name=f"C1{d}", tag=f"C1{d}") for d in "fb"}
            h00 = state.tile([H, B], BF16, name="h00", tag="h00")
            nc.vector.memset(h00[:], 0.0)
            for d in "fb":
                nc.vector.memset(C0[d][:], 0.0)
                nc.vector.memset(C1[d][:], 0.0)

            xg1_dram = {d: dram.tile([H, SPAN1 * 4 * B], BF16, name=f"xg1d{d}", tag=f"xg1d{d}") for d in "fb"}

            # ---------------- generic LSTM step ----------------
            # Gate col order in psum/t-tile: [o | i | f | g]*B; cell state
            # C=2c lives in t-tile cols 4B:5B (written by the PREVIOUS step's
            # c-update into THIS step's tile, so (1+ti)*tg and (1+tf)*C fuse
            # into one scalar_tensor_tensor over [i|f] x [g|C]).
            def lstm_prefetch(tag, inproj, first=False):
                ps_pool = psA if tag.endswith("f") else psB
                g_ps = ps_pool.tile([H, 4 * B], F32, name="g" + tag, tag="g" + tag[-1],
                                    bufs=3)
                inproj(g_ps)
                t_t = tpool.tile([H, 5 * B], F32, name="t" + tag, tag="t" + tag, bufs=4)
                if first:
                    nc.vector.memset(t_t[:, 4 * B:5 * B], 0.0)
                return g_ps, t_t

            def lstm_step(tag, whh_t, hprev, yout_slice, cur, nxt):
                g_ps, T = cur
                Tn = nxt[1]
                for g in range(4):
                    nc.tensor.matmul(g_ps[:, g * B:(g + 1) * B],
                                     whh_t[:, g * H:(g + 1) * H], hprev,
                                     start=False, stop=True,
                                     skip_group_check=True)
                nc.scalar.activation(T[:, B:4 * B], g_ps[:, B:4 * B], ACT_TANH)
                nc.scalar.activation(T[:, 0:B], g_ps[:, 0:B], ACT_TANH)
                scr = vpool.tile([H, 2 * B], F32, name="s" + tag, tag="s" + tag)
                # scr = [(1+ti)*tg | (1+tf)*C] = [Bv | A]
                nc.vector.scalar_tensor_tensor(scr[:], T[:, B:3 * B], 1.0,
                                               T[:, 3 * B:5 * B], AluOp.add, AluOp.mult)
                nc.vector.scalar_tensor_tensor(Tn[:, 4 * B:5 * B], scr[:, B:2 * B], 0.5,
                                               scr[:, 0:B], AluOp.mult, AluOp.add)
                tc_t = vpool.tile([H, B], F32, name="c" + tag, tag="c" + tag)
                nc.scalar.activation(tc_t[:], Tn[:, 4 * B:5 * B], ACT_TANH, scale=0.5)
                nc.vector.scalar_tensor_tensor(yout_slice, T[:, 0:B], 1.0, tc_t[:],
                                               AluOp.add, AluOp.mult)

            # ---------------- layer 0 (fused input projection) ----------------
            def l0_inproj(dirn, slot):
                def fn(g_ps):
                    for g in range(4):
                        nc.tensor.matmul(g_ps[:, g * B:(g + 1) * B],
                                         wihT0_t[dirn][:, g * H:(g + 1) * H],
                                         xaug_t[:, slot * B:(slot + 1) * B],
                                         start=True, stop=False,
                                         skip_group_check=True)
                return fn

            pend0 = {}
            for step in range(SPAN0 + 1):
                sf = step                 # fwd slot, ascending
                sb = SPAN0 - 1 - step     # bwd slot, descending
                if step < SPAN0:
                    pend0[("f", step)] = lstm_prefetch("0f", l0_inproj("f", sf), first=(step == 0))
                    pend0[("b", step)] = lstm_prefetch("0b", l0_inproj("b", sb), first=(step == 0))
                else:
                    pend0[("f", step)] = lstm_prefetch("0f", lambda ps: None)
                    pend0[("b", step)] = lstm_prefetch("0b", lambda ps: None)
                if step >= 1:
                    pf, pb = step - 1, SPAN0 - step
                    hp_f = h00[:] if pf == 0 else y0["f"][:, (pf - 1) * B:pf * B]
                    hp_b = h00[:] if pf == 0 else y0["b"][:, (pb + 1) * B:(pb + 2) * B]
                    lstm_step("0f", whhT0_t["f"], hp_f,
                              y0["f"][:, pf * B:(pf + 1) * B],
                              pend0.pop(("f", pf)), pend0[("f", step)])
                    lstm_step("0b", whhT0_t["b"], hp_b,
                              y0["b"][:, pb * B:(pb + 1) * B],
                              pend0.pop(("b", step - 1)), pend0[("b", step)])

            # ---------------- layer-1 projection -> DRAM ----------------
            # xg1_d covers slots [lo, lo+SPAN1); col layout per slot: [i|f|g|o]*B
            CH = 512                      # psum cols per chunk = 8 slots
            SLOTS_PER_CH = CH // B
            NCH = SPAN1 * B // CH         # 14 chunks
            # chain f consumes slots [0, SPAN1); chain b consumes [WARM, SPAN0)
            proj_lo = {"f": 0, "b": WARM}

            def l1_proj_chunk(dirn, j):
                lo = proj_lo[dirn]
                col0 = (lo + j * SLOTS_PER_CH) * B          # into y0/ctl tiles
                st = stage_pool.tile([H, SLOTS_PER_CH * 4 * B], BF16, name="st", tag="st")
                for g in range(4):
                    p = psP.tile([H, CH], F32, name="pp", tag="pp")
                    nc.tensor.matmul(p[:], wih1Ta_t[dirn][:, g * H:(g + 1) * H],
                                     y0["f"][:, col0:col0 + CH], start=True, stop=False)
                    nc.tensor.matmul(p[:], wih1Tb_t[dirn][:, g * H:(g + 1) * H],
                                     y0["b"][:, col0:col0 + CH], start=False, stop=False)
                    nc.tensor.matmul(p[:], ctlT1_t[dirn][:, g * H:(g + 1) * H],
                                     ctl1_t[:, col0:col0 + CH], start=False, stop=True)
                    # scatter gate g into per-slot interleaved layout
                    st3 = st[:].rearrange("h (s c) -> h s c", s=SLOTS_PER_CH)
                    nc.any.tensor_copy(st3[:, :, g * B:(g + 1) * B],
                                       p[:].rearrange("h (s b) -> h s b", b=B))
                nc.sync.dma_start(
                    out=xg1_dram[dirn][:, j * SLOTS_PER_CH * 4 * B:(j + 1) * SLOTS_PER_CH * 4 * B],
                    in_=st[:])

            # ---------------- layer 1 recurrence (proj interleaved) ----------------
            # xg1 fetch tiles: 8 slots per fetch, chain f ascending, chain b descending
            NFETCH = SPAN1 // SLOTS_PER_CH
            fet = {"f": [None] * NFETCH, "b": [None] * NFETCH}

            def get_fetch(dirn, k):
                if fet[dirn][k] is None:
                    ft = xg1fetch.tile([H, SLOTS_PER_CH * 4 * B], BF16, name="x" + dirn, tag="x" + dirn)
                    if dirn == "f":     # fetch k covers local idx [8k, 8k+8)
                        c0 = k * SLOTS_PER_CH * 4 * B
                    else:               # fetch k covers local idx [SPAN1-8(k+1), SPAN1-8k)
                        c0 = (SPAN1 - (k + 1) * SLOTS_PER_CH) * 4 * B
                    nc.sync.dma_start(out=ft[:], in_=xg1_dram[dirn][:, c0:c0 + SLOTS_PER_CH * 4 * B])
                    fet[dirn][k] = ft
                return fet[dirn][k]

            def l1_inproj(xs):
                def fn(g_ps):
                    nc.tensor.matmul(g_ps[:], idn_t[:], xs, start=True, stop=False,
                                     skip_group_check=True)
                return fn

            # chunks needed first: f ascending from 0, b descending from NCH-1
            l1_proj_chunk("f", 0)
            l1_proj_chunk("b", NCH - 1)

            pend1 = {}
            for step in range(SPAN1 + 1):
                if step < SPAN1:
                    if step % SLOTS_PER_CH == 0:
                        k = step // SLOTS_PER_CH
                        if k + 1 < NCH:
                            l1_proj_chunk("f", k + 1)
                        if NCH - 2 - k >= 0:
                            l1_proj_chunk("b", NCH - 2 - k)
                    kf = step // SLOTS_PER_CH
                    ff = get_fetch("f", kf)
                    xs_f = ff[:, (step % SLOTS_PER_CH) * 4 * B:((step % SLOTS_PER_CH) + 1) * 4 * B]
                    fb_ = get_fetch("b", kf)
                    within_b = (SLOTS_PER_CH - 1) - (step % SLOTS_PER_CH)
                    xs_b = fb_[:, within_b * 4 * B:(within_b + 1) * 4 * B]
                    pend1[("f", step)] = lstm_prefetch("1f", l1_inproj(xs_f), first=(step == 0))
                    pend1[("b", step)] = lstm_prefetch("1b", l1_inproj(xs_b), first=(step == 0))
                else:
                    pend1[("f", step)] = lstm_prefetch("1f", lambda ps: None)
                    pend1[("b", step)] = lstm_prefetch("1b", lambda ps: None)
                if step >= 1:
                    pf = step - 1
                    pb = SPAN1 - 1 - pf
                    hp_f = h00[:] if pf == 0 else y1["f"][:, (pf - 1) * B:pf * B]
                    hp_b = h00[:] if pf == 0 else y1["b"][:, (pb + 1) * B:(pb + 2) * B]
                    lstm_step("1f", whhT1_t["f"], hp_f,
                              y1["f"][:, pf * B:(pf + 1) * B],
                              pend1.pop(("f", pf)), pend1[("f", step)])
                    lstm_step("1b", whhT1_t["b"], hp_b,
                              y1["b"][:, pb * B:(pb + 1) * B],
                              pend1.pop(("b", pf)), pend1[("b", step)])

            # ---------------- classifier (window slots only) ----------------
            # window tokens: slot s in [WARM, WARM+WIN)
            #   y1f idx = s        -> cols [WARM*B, (WARM+WIN)*B)
            #   y1b idx = s - WARM -> cols [0, WIN*B)
            # ones: ctl1 row0 cols [WARM*B ...)
            NTOK = WIN * B                      # 4096 columns
            h1 = [clssb.tile([H, NTOK], BF16, name="h1a", tag="h1a", bufs=1), clssb.tile([H, NTOK], BF16, name="h1b", tag="h1b", bufs=1)]
            for c0 in range(0, NTOK, CH):
                for m in range(2):
                    p = psP.tile([H, CH], F32, name="pc", tag="pp")
                    nc.tensor.matmul(p[:], w1Ta_t[:, m * H:(m + 1) * H],
                                     y1["f"][:, WARM * B + c0:WARM * B + c0 + CH],
                                     start=True, stop=False)
                    nc.tensor.matmul(p[:], w1Tb_t[:, m * H:(m + 1) * H],
                                     y1["b"][:, c0:c0 + CH], start=False, stop=False)
                    nc.tensor.matmul(p[:], b1row_t[:, m * H:(m + 1) * H],
                                     ctl1_t[0:1, WARM * B + c0:WARM * B + c0 + CH],
                                     start=False, stop=True)
                    nc.scalar.activation(h1[m][:, c0:c0 + CH], p[:], ACT_RELU)

            # final GEMM transposed: out[tok, d] (tokens on partitions)
            for c0 in range(0, NTOK, H):
                p = psP.tile([H, D], F32, name="po", tag="pp")
                nc.tensor.matmul(p[:], h1[0][:, c0:c0 + H], w2Ta_t[:], start=True, stop=False)
                nc.tensor.matmul(p[:], h1[1][:, c0:c0 + H], w2Tb_t[:], start=False, stop=False)
                nc.tensor.matmul(p[:], ctl1_t[0:1, WARM * B + c0:WARM * B + c0 + H],
                                 b2row_t[:], start=False, stop=True)
                o_t = clssb.tile([H, D], F32, name="ot", tag="ot")
                nc.scalar.activation(o_t[:], p[:], ACT_TANH)
                nc.sync.dma_start(out=out[c0:c0 + H, :], in_=o_t[:])

    nc.compile()
    return nc


# ======================= host side =======================

def _prep_weights(inp):
    """Returns dict of np arrays shared by all cores (bf16).

    Gate row-blocks reordered from reference [i,f,g,o] to device [o,i,f,g];
    i,f,o rows scaled 0.5 (one-tanh trick)."""
    H_ = H
    sr = np.full((4 * H_, 1), 0.5, np.float32)
    sr[2 * H_:3 * H_] = 1.0

    def reorder(a):           # rows [i,f,g,o] -> [o,i,f,g]
        return np.concatenate([a[3 * H_:], a[:H_], a[H_:2 * H_], a[2 * H_:3 * H_]], 0)

    w = {}
    for d, tag in (("f", "0"), ("b", "1")):
        Wih, Whh = inp[f"Wih0{tag}"], inp[f"Whh0{tag}"]
        bias = inp[f"bih0{tag}"] + inp[f"bhh0{tag}"]
        w[f"wihT0{d}"] = reorder(np.concatenate([Wih * sr, (bias[:, None] * sr)], 1)).T.astype(bf16)
        w[f"whhT0{d}"] = reorder(Whh * sr * 0.5).T.astype(bf16)
        Wih1, Whh1 = inp[f"Wih1{tag}"], inp[f"Whh1{tag}"]
        bias1 = reorder((inp[f"bih1{tag}"] + inp[f"bhh1{tag}"])[:, None] * sr).T
        w[f"whhT1{d}"] = reorder(Whh1 * sr * 0.5).T.astype(bf16)
        w[f"wih1Ta{d}"] = reorder(Wih1[:, :H] * sr * 0.5).T.astype(bf16)
        w[f"wih1Tb{d}"] = reorder(Wih1[:, H:] * sr * 0.5).T.astype(bf16)
        padkill = np.zeros((1, 4 * H), np.float32)
        padkill[0, H:2 * H] = PADKILL      # i-gate block (device order [o,i,f,g])
        w[f"ctlT1{d}"] = np.concatenate([bias1, padkill], 0).astype(bf16)
    w["idn"] = np.eye(H, dtype=np.float32).astype(bf16)
    w["w1Ta"] = (0.5 * inp["W1"][:, :H]).T.astype(bf16)
    w["w1Tb"] = (0.5 * inp["W1"][:, H:]).T.astype(bf16)
    w["b1row"] = inp["b1"][None, :].astype(bf16)
    w["w2Ta"] = inp["W2"][:, :H].T.astype(bf16)
    w["w2Tb"] = inp["W2"][:, H:].T.astype(bf16)
    w["b2row"] = inp["b2"][None, :].astype(bf16)
    return w


def _per_core_inputs(x, q):
    """x: [B, T, D] f32.  Builds xaug [65, SPAN0*B] and ctl1 [2, SPAN0*B]."""
    t0 = WIN * q - WARM
    xaug = np.zeros((D + 1, SPAN0 * B), np.float32)
    ctl = np.zeros((2, SPAN0 * B), np.float32)
    for s in range(SPAN0):
        t = t0 + s
        sl = slice(s * B, (s + 1) * B)
        if 0 <= t < T:
            xaug[:D, sl] = x[:, t, :].T
            xaug[D, sl] = 1.0
            ctl[0, sl] = 1.0
        else:
            ctl[1, sl] = 1.0
    return xaug.astype(bf16), ctl.astype(bf16)


def _get_program():
    if "nc" not in _CACHE:
        _CACHE["nc"] = _build_program()
    return _CACHE["nc"]


def _run(inputs, trace=False):
    inp = {k: np.asarray(v) for k, v in inputs.items()}
    nc = _get_program()
    w = _prep_weights(inp)
    x = inp["x"].astype(np.float32)
    in_maps = []
    for q in range(NC):
        xaug, ctl = _per_core_inputs(x, q)
        m = dict(w)
        m["xaug"] = xaug
        m["ctl1"] = ctl
        in_maps.append(m)
    res = run_bass_kernel_spmd(nc, in_maps, list(range(NC)), trace=trace)
    outp = np.zeros((B, T, D), np.float32)
    for q in range(NC):
        o = res.results[q]["out"].reshape(WIN, B, D)        # [tok, b, d]
        outp[:, WIN * q:WIN * (q + 1), :] = o.transpose(1, 0, 2)
    return outp, res


def kernel(**inputs):
    out, _ = _run(inputs, trace=False)
    return out
